# revision 8
# baseline (speedup 1.0000x reference)
"""AttentionBlock (GroupNorm + single-head self-attention + residual) on
8 TRN2 NeuronCores.

Sharding: data-parallel over batch (4 images) x 2-way sequence-parallel
over query tokens => 8 cores, zero collectives. Each core receives one
full image x[b] [C=512, N=4096] (token columns rotated so that its own
2048 query tokens sit in columns 0..2047), computes GroupNorm + K/V over
all 4096 tokens (K/V duplicated across the 2 cores of a batch pair --
cheaper than an all-gather at this size), Q / attention / proj / residual
for its 2048 queries, and returns y [512, 2048].

On-chip layout ("channels on partitions"):
  t  = groupnorm(x)            [c, n]  bf16   (4 tiles [128, 4096])
  Q  = wq @ t  (+bq)           [d, nq] bf16
  K  = wk @ t  (+bk)           [d, m]  bf16
  V  = (t^T @ wvT) (+bv)       [m, d]  bf16   (computed directly transposed)
  S^T[m, nq] = K^T Q           (PE, accumulated over 4 d-tiles)
  E  = exp(S^T / sqrt(C))      (ScalarE, PSUM->SBUF, bf16)
  L[nq]  = ones^T @ E          (PE accumulate over 32 m-tiles)
  O_u[d, nq] = V^T @ E         (PE accumulate over 32 m-tiles)
  O  = O_u * (1/L)             (K=1 broadcast matmul + DVE mul)
  y  = wp @ O + bp + x         [c, nq] f32 -> DMA out

Softmax skips the max-subtraction: logits are ~N(0,1) by construction
(1/sqrt(C) scaling of unit-variance q,k), so exp() is safely bounded.
"""

import sys

for _p in ("/opt/trn_rl_repo", "/opt/pypackages"):
    if _p not in sys.path:
        sys.path.append(_p)

import numpy as np

import concourse.bass as bass
import concourse.tile as tile
from concourse import mybir
from concourse.bass_utils import run_bass_kernel_spmd
from concourse.vector_clock import ScopedClock

# ----------------------------------------------------------------------
# Problem constants (nn_AttentionBlock_24764781429183)
B, C, H, W = 4, 512, 64, 64
N = H * W              # 4096 tokens
NQ = N // 2            # 2048 query tokens per core
GROUPS = 32
GSIZE = C // GROUPS    # 16 channels per group
EPS = 1e-5
SCALE = 1.0 / float(np.sqrt(C))
CT = C // 128          # 4 channel tiles
MT = N // 128          # 32 key tiles
FB = 512               # matmul free-dim block
QB = NQ // FB          # 4 query blocks
NB = N // FB           # 8 token blocks

F32 = mybir.dt.float32
BF16 = mybir.dt.bfloat16
IDENT = mybir.ActivationFunctionType.Identity
EXP = mybir.ActivationFunctionType.Exp
SQRT = mybir.ActivationFunctionType.Sqrt


# ----------------------------------------------------------------------
# This container's walrus build rejects >1 semaphore wait on one CTRL
# (Drain) instruction; split the Tile end-of-kernel drain waits across
# one-nop-per-wait instead.
def _patched_drain_and_barrier(self, tick_clock, wait_clock):
    nc = self.nc
    probe = nc.sync.nop(nofuse=True)
    wait_clock.add_sem_waits(probe.ins, ScopedClock({None: tick_clock.global_clock}))
    sync_info = probe.ins.sync_info
    waits = list(sync_info.on_wait or []) if sync_info is not None else []
    if sync_info is not None and len(waits) > 1:
        sync_info.on_wait = waits[:1]
        for w in waits[1:]:
            n = nc.sync.nop(nofuse=True)
            if n.ins.sync_info is None:
                n.ins.sync_info = type(sync_info)(on_wait=[w], on_update=[])
            else:
                n.ins.sync_info.on_wait = [w]
    nc.sync.drain()
    nc.all_engine_barrier()
    assert self.sems is not None
    popped = nc._tile_sem_poison_stack.pop()
    assert popped is self._sem_poison
    nc.clear_and_free_semaphores(list(self.sems.allocated().values()))
    nc.all_engine_barrier()


tile.TileContext._drain_and_barrier = _patched_drain_and_barrier


def _split_multi_waits(nc: bass.Bass, maxw: int = 1) -> None:
    """Walrus in this container rejects instructions carrying more than one
    semaphore wait. Hoist extra waits onto same-engine no-ops inserted
    right before the instruction (engine streams execute in block order,
    so the waits still gate the instruction)."""
    ctr = 0
    for fn in nc.m.functions:
        for bb in fn.blocks:
            out = []
            changed = False
            for inst in bb.instructions:
                si = inst.sync_info
                waits = list(si.on_wait) if (si is not None and si.on_wait) else []
                if len(waits) > maxw and inst.engine != mybir.EngineType.Unassigned:
                    keep = waits[-maxw:]
                    for i in range(0, len(waits) - maxw, maxw):
                        nop = mybir.InstNoOp(name=f"waitsplit-{ctr}")
                        ctr += 1
                        nop.engine = inst.engine
                        nop.sync_info = mybir.SyncInfo(
                            on_wait=waits[i : i + maxw], on_update=[]
                        )
                        out.append(nop)
                    si.on_wait = keep
                    inst.sync_info = si
                    changed = True
                out.append(inst)
            if changed:
                bb.instructions = out


# ----------------------------------------------------------------------
def _build_nc() -> bass.Bass:
    nc = bass.Bass()

    x_ext = nc.declare_dram_parameter("x", [C, N], F32, isOutput=False)
    w_ext = {
        k: nc.declare_dram_parameter(k, [C, C], F32, isOutput=False)
        for k in ("wqT", "wkT", "wvT", "wpT")
    }
    b_ext = {
        k: nc.declare_dram_parameter(k, [C], F32, isOutput=False)
        for k in ("bq", "bk", "bv", "bp", "gnw", "gnb")
    }
    ind16_ext = nc.declare_dram_parameter("ind16", [128, 8], F32, isOutput=False)
    indT8_ext = nc.declare_dram_parameter("indT8", [8, 128], F32, isOutput=False)
    out_ext = nc.declare_dram_parameter("out", [C, NQ], F32, isOutput=True)

    with tile.TileContext(nc) as tc:
        _body(nc, tc, x_ext, w_ext, b_ext, ind16_ext, indT8_ext, out_ext)
    _split_multi_waits(nc)
    return nc


def _body(nc, tc, x_ext, w_ext, b_ext, ind16_ext, indT8_ext, out_ext):
    from contextlib import ExitStack

    ctx = ExitStack()
    with ctx:
        const = ctx.enter_context(tc.tile_pool(name="const", bufs=1))
        big = ctx.enter_context(tc.tile_pool(name="big", bufs=1))
        mm_psum = ctx.enter_context(tc.tile_pool(name="mm_psum", bufs=2, space="PSUM"))

        # ---- constants -------------------------------------------------
        ind16 = const.tile([128, 8], F32, tag="ind16")
        nc.sync.dma_start(out=ind16, in_=ind16_ext[:])
        indT8 = const.tile([8, 128], F32, tag="indT8")
        nc.sync.dma_start(out=indT8, in_=indT8_ext[:])

        ones_bf = const.tile([128, 1], BF16, tag="ones_bf")
        nc.vector.memset(ones_bf, 1.0)
        ones_row = const.tile([1, 128], F32, tag="ones_row")
        nc.vector.memset(ones_row, 1.0)

        bias_sb = {}
        for k in ("bq", "bk", "bp", "gnw", "gnb"):
            t = const.tile([128, CT], F32, tag=f"bias_{k}")
            nc.sync.dma_start(out=t, in_=b_ext[k][:].rearrange("(o p) -> p o", p=128))
            bias_sb[k] = t
        # bv broadcast along partitions: [512] -> [128, 512]
        bvb = const.tile([128, C], F32, tag="bvb")
        bv_ap = b_ext["bv"][:]
        nc.sync.dma_start(
            out=bvb,
            in_=bass.AP(tensor=bv_ap.tensor, offset=bv_ap.offset, ap=[[0, 128], *bv_ap.ap]),
        )

        # ---- weights: f32 staging -> bf16 [128, CT, C] -----------------
        w_bf = {}
        with tc.tile_pool(name="wstage", bufs=2) as wstage:
            for k in ("wqT", "wkT", "wvT", "wpT"):
                wb = big.tile([128, CT, C], BF16, tag=f"wbf_{k}")
                for ct in range(CT):
                    ws = wstage.tile([128, C], F32, tag="wstage")
                    nc.sync.dma_start(
                        out=ws, in_=w_ext[k][ct * 128 : (ct + 1) * 128, :]
                    )
                    nc.vector.tensor_copy(out=wb[:, ct, :], in_=ws)
                w_bf[k] = wb

        # ---- persistent activations ------------------------------------
        t_big = big.tile([128, CT, N], BF16, tag="t_big")
        xb_big = big.tile([128, CT, NQ], F32, tag="xb_big")

        # ---- phase 1: GroupNorm ----------------------------------------
        with (
            tc.tile_pool(name="gn", bufs=2) as gn_pool,
            tc.tile_pool(name="gn_small", bufs=4) as small,
            tc.tile_pool(name="gn_psum", bufs=2, space="PSUM") as gn_psum,
        ):
            for ct in range(CT):
                xf = gn_pool.tile([128, N], F32, tag="xf")
                nc.sync.dma_start(out=xf, in_=x_ext[ct * 128 : (ct + 1) * 128, :])

                xf3 = xf.rearrange("p (c f) -> p c f", f=512)
                stats6 = small.tile([128, N // 512, 6], F32, tag="stats6")
                for c in range(N // 512):
                    nc.vector.bn_stats(out=stats6[:, c, :], in_=xf3[:, c, :])
                mv = small.tile([128, 2], F32, tag="mv")
                nc.vector.bn_aggr(out=mv, in_=stats6)

                # stats2 = [mean_c, E[x^2]_c]
                stats2 = small.tile([128, 2], F32, tag="stats2")
                nc.vector.tensor_copy(out=stats2[:, 0:1], in_=mv[:, 0:1])
                nc.vector.tensor_mul(stats2[:, 1:2], mv[:, 0:1], mv[:, 0:1])
                nc.vector.tensor_add(stats2[:, 1:2], stats2[:, 1:2], mv[:, 1:2])

                # group aggregation: [8, 2] = (1/16) * sum over 16-ch groups
                gpsum = gn_psum.tile([8, 2], F32, tag="gpsum")
                nc.tensor.matmul(gpsum, lhsT=ind16, rhs=stats2, start=True, stop=True)

                gss = small.tile([8, 2], F32, tag="gss")
                nc.vector.tensor_copy(out=gss, in_=gpsum)
                g_sb = small.tile([8, 2], F32, tag="g_sb")
                nc.vector.tensor_copy(out=g_sb[:, 0:1], in_=gss[:, 0:1])
                msqg = small.tile([8, 1], F32, tag="msqg")
                nc.vector.tensor_mul(msqg, gss[:, 0:1], gss[:, 0:1])
                epsm = small.tile([8, 1], F32, tag="epsm")
                nc.vector.tensor_scalar(
                    epsm,
                    msqg,
                    -1.0,
                    EPS,
                    op0=mybir.AluOpType.mult,
                    op1=mybir.AluOpType.add,
                )
                stdg = small.tile([8, 1], F32, tag="stdg")
                nc.scalar.activation(stdg, gss[:, 1:2], SQRT, bias=epsm, scale=1.0)
                nc.vector.reciprocal(out=g_sb[:, 1:2], in_=stdg)

                # broadcast per-group -> per-channel: [128, 2] = indT8^T @ g_sb
                ppsum = gn_psum.tile([128, 2], F32, tag="ppsum")
                nc.tensor.matmul(ppsum, lhsT=indT8, rhs=g_sb, start=True, stop=True)

                alpha = small.tile([128, 1], F32, tag="alpha")
                nc.vector.tensor_mul(alpha, ppsum[:, 1:2], bias_sb["gnw"][:, ct : ct + 1])
                beta = small.tile([128, 1], F32, tag="beta")
                nc.vector.tensor_mul(beta, ppsum[:, 0:1], alpha)
                nc.vector.tensor_sub(beta, bias_sb["gnb"][:, ct : ct + 1], beta)

                # t = alpha*x + beta (bf16); xb = x + bp (residual staging)
                nc.scalar.activation(t_big[:, ct, :], xf, IDENT, bias=beta, scale=alpha)
                nc.scalar.activation(
                    xb_big[:, ct, :],
                    xf[:, 0:NQ],
                    IDENT,
                    bias=bias_sb["bp"][:, ct : ct + 1],
                    scale=1.0,
                )

        # ---- phase 2: Q / K / V projections ----------------------------
        q_big = big.tile([128, CT, NQ], BF16, tag="q_big")
        k_big = big.tile([128, CT, N], BF16, tag="k_big")
        v_big = big.tile([128, MT, C], BF16, tag="v_big")

        for dt in range(CT):
            for nb in range(QB):
                qp = mm_psum.tile([128, FB], F32, tag="mm")
                for ct in range(CT):
                    nc.tensor.matmul(
                        qp,
                        lhsT=w_bf["wqT"][:, ct, dt * 128 : (dt + 1) * 128],
                        rhs=t_big[:, ct, nb * FB : (nb + 1) * FB],
                        start=(ct == 0),
                        stop=(ct == CT - 1),
                    )
                nc.scalar.activation(
                    q_big[:, dt, nb * FB : (nb + 1) * FB],
                    qp,
                    IDENT,
                    bias=bias_sb["bq"][:, dt : dt + 1],
                    scale=1.0,
                )
        for dt in range(CT):
            for nb in range(NB):
                kp = mm_psum.tile([128, FB], F32, tag="mm")
                for ct in range(CT):
                    nc.tensor.matmul(
                        kp,
                        lhsT=w_bf["wkT"][:, ct, dt * 128 : (dt + 1) * 128],
                        rhs=t_big[:, ct, nb * FB : (nb + 1) * FB],
                        start=(ct == 0),
                        stop=(ct == CT - 1),
                    )
                nc.scalar.activation(
                    k_big[:, dt, nb * FB : (nb + 1) * FB],
                    kp,
                    IDENT,
                    bias=bias_sb["bk"][:, dt : dt + 1],
                    scale=1.0,
                )
        for mt in range(MT):
            vp = mm_psum.tile([128, C], F32, tag="mm")
            for ct in range(CT):
                nc.tensor.matmul(
                    vp,
                    lhsT=t_big[:, ct, mt * 128 : (mt + 1) * 128],
                    rhs=w_bf["wvT"][:, ct, :],
                    start=(ct == 0),
                    stop=(ct == CT - 1),
                )
            nc.vector.tensor_add(v_big[:, mt, :], vp, bvb)

        # ---- phase 3: attention + proj + residual ----------------------
        with (
            tc.tile_pool(name="o_psum", bufs=1, space="PSUM") as o_psum,
            tc.tile_pool(name="l_psum", bufs=1, space="PSUM") as l_psum,
            tc.tile_pool(name="rb_psum", bufs=1, space="PSUM") as rb_psum,
            tc.tile_pool(name="e_pool", bufs=4) as e_pool,
            tc.tile_pool(name="att_sb", bufs=2) as att_sb,
            tc.tile_pool(name="y_pool", bufs=4) as y_pool,
        ):
            for qb in range(QB):
                qs = slice(qb * FB, (qb + 1) * FB)
                op = [
                    o_psum.tile([128, FB], F32, tag=f"o{dc}", name=f"o_{qb}_{dc}")
                    for dc in range(CT)
                ]
                lp = l_psum.tile([1, FB], F32, tag="l")

                for mt in range(MT):
                    sp = mm_psum.tile([128, FB], F32, tag="mm")
                    for dt in range(CT):
                        nc.tensor.matmul(
                            sp,
                            lhsT=k_big[:, dt, mt * 128 : (mt + 1) * 128],
                            rhs=q_big[:, dt, qs],
                            start=(dt == 0),
                            stop=(dt == CT - 1),
                        )
                    et = e_pool.tile([128, FB], BF16, tag="et")
                    nc.scalar.activation(et, sp, EXP, bias=0.0, scale=SCALE)
                    nc.tensor.matmul(
                        lp, lhsT=ones_bf, rhs=et, start=(mt == 0), stop=(mt == MT - 1)
                    )
                    for dc in range(CT):
                        nc.tensor.matmul(
                            op[dc],
                            lhsT=v_big[:, mt, dc * 128 : (dc + 1) * 128],
                            rhs=et,
                            start=(mt == 0),
                            stop=(mt == MT - 1),
                        )

                recip = att_sb.tile([1, FB], F32, tag="recip")
                nc.vector.reciprocal(out=recip, in_=lp)
                rbp = rb_psum.tile([128, FB], F32, tag="rb")
                nc.tensor.matmul(rbp, lhsT=ones_row, rhs=recip, start=True, stop=True)
                rb_sb = att_sb.tile([128, FB], F32, tag="rb_sb")
                nc.vector.tensor_copy(out=rb_sb, in_=rbp)

                o_sb = att_sb.tile([128, CT, FB], BF16, tag="o_sb")
                for dc in range(CT):
                    nc.vector.tensor_mul(o_sb[:, dc, :], op[dc], rb_sb)

                for pt in range(CT):
                    pj = mm_psum.tile([128, FB], F32, tag="mm")
                    for dc in range(CT):
                        nc.tensor.matmul(
                            pj,
                            lhsT=w_bf["wpT"][:, dc, pt * 128 : (pt + 1) * 128],
                            rhs=o_sb[:, dc, :],
                            start=(dc == 0),
                            stop=(dc == CT - 1),
                        )
                    y_tile = y_pool.tile([128, FB], F32, tag="y")
                    nc.vector.tensor_add(y_tile, pj, xb_big[:, pt, qs])
                    nc.sync.dma_start(
                        out=out_ext[pt * 128 : (pt + 1) * 128, qs], in_=y_tile
                    )


_NC_CACHE = None


def _get_nc():
    global _NC_CACHE
    if _NC_CACHE is None:
        _NC_CACHE = _build_nc()
    return _NC_CACHE


def _make_indicators():
    p = np.arange(128)
    ind16 = np.zeros((128, 8), np.float32)
    ind16[p, p // GSIZE] = 1.0 / GSIZE
    indT8 = np.zeros((8, 128), np.float32)
    indT8[p // GSIZE, p] = 1.0
    return ind16, indT8


def kernel(**inputs) -> np.ndarray:
    x = np.ascontiguousarray(np.asarray(inputs["x"], dtype=np.float32))
    assert x.shape == (B, C, H, W), x.shape
    xf = x.reshape(B, C, N)

    common = {}
    for name, key in (("wqT", "wq"), ("wkT", "wk"), ("wvT", "wv"), ("wpT", "wp")):
        common[name] = np.ascontiguousarray(np.asarray(inputs[key], np.float32).T)
    for key in ("bq", "bk", "bv", "bp"):
        common[key] = np.ascontiguousarray(np.asarray(inputs[key], np.float32))
    common["gnw"] = np.ascontiguousarray(np.asarray(inputs["gn_w"], np.float32))
    common["gnb"] = np.ascontiguousarray(np.asarray(inputs["gn_b"], np.float32))
    common["ind16"], common["indT8"] = _make_indicators()

    in_maps = []
    for core in range(8):
        b, h = core // 2, core % 2
        if h == 0:
            xc = xf[b]
        else:
            # rotate so this core's query tokens land in columns 0..NQ-1
            xc = np.concatenate([xf[b][:, NQ:], xf[b][:, :NQ]], axis=1)
        in_maps.append({"x": np.ascontiguousarray(xc), **common})

    nc = _get_nc()
    res = run_bass_kernel_spmd(nc, in_maps, core_ids=list(range(8)))

    out = np.empty((B, C, N), np.float32)
    for core in range(8):
        b, h = core // 2, core % 2
        out[b][:, h * NQ : (h + 1) * NQ] = res.results[core]["out"]
    return out.reshape(B, C, H, W)


# revision 11
# speedup vs baseline: 1.0171x; 1.0171x over previous
"""AttentionBlock (GroupNorm + single-head self-attention + residual) on
8 TRN2 NeuronCores.

Sharding: data-parallel over batch (4 images) x 2-way sequence-parallel
over query tokens => 8 cores, zero collectives. Each core receives one
full image x[b] [C=512, N=4096] (token columns rotated so that its own
2048 query tokens sit in columns 0..2047), computes GroupNorm + K/V over
all 4096 tokens (K/V duplicated across the 2 cores of a batch pair --
cheaper than an all-gather at this size), Q / attention / proj / residual
for its 2048 queries, and returns y [512, 2048].

On-chip layout ("channels on partitions"):
  t  = groupnorm(x)            [c, n]  bf16   (4 tiles [128, 4096])
  Q  = wq @ t  (+bq)           [d, nq] bf16
  K  = wk @ t  (+bk)           [d, m]  bf16
  V  = (t^T @ wvT) (+bv)       [m, d]  bf16   (computed directly transposed)
  S^T[m, nq] = K^T Q           (PE, accumulated over 4 d-tiles)
  E  = exp(S^T / sqrt(C))      (ScalarE, PSUM->SBUF, bf16)
  L[nq]  = ones^T @ E          (PE accumulate over 32 m-tiles)
  O_u[d, nq] = V^T @ E         (PE accumulate over 32 m-tiles)
  O  = O_u * (1/L)             (K=1 broadcast matmul + DVE mul)
  y  = wp @ O + bp + x         [c, nq] f32 -> DMA out

Softmax skips the max-subtraction: logits are ~N(0,1) by construction
(1/sqrt(C) scaling of unit-variance q,k), so exp() is safely bounded.
"""

import sys

for _p in ("/opt/trn_rl_repo", "/opt/pypackages"):
    if _p not in sys.path:
        sys.path.append(_p)

import numpy as np

import concourse.bass as bass
import concourse.tile as tile
from concourse import mybir
from concourse.bass_utils import run_bass_kernel_spmd
from concourse.vector_clock import ScopedClock

# ----------------------------------------------------------------------
# Problem constants (nn_AttentionBlock_24764781429183)
B, C, H, W = 4, 512, 64, 64
N = H * W              # 4096 tokens
NQ = N // 2            # 2048 query tokens per core
GROUPS = 32
GSIZE = C // GROUPS    # 16 channels per group
EPS = 1e-5
SCALE = 1.0 / float(np.sqrt(C))
CT = C // 128          # 4 channel tiles
MT = N // 128          # 32 key tiles
FB = 512               # matmul free-dim block
QB = NQ // FB          # 4 query blocks
NB = N // FB           # 8 token blocks

F32 = mybir.dt.float32
BF16 = mybir.dt.bfloat16
IDENT = mybir.ActivationFunctionType.Identity
EXP = mybir.ActivationFunctionType.Exp
SQRT = mybir.ActivationFunctionType.Sqrt


# ----------------------------------------------------------------------
# This container's walrus build rejects >1 semaphore wait on one CTRL
# (Drain) instruction; split the Tile end-of-kernel drain waits across
# one-nop-per-wait instead.
def _patched_drain_and_barrier(self, tick_clock, wait_clock):
    nc = self.nc
    probe = nc.sync.nop(nofuse=True)
    wait_clock.add_sem_waits(probe.ins, ScopedClock({None: tick_clock.global_clock}))
    sync_info = probe.ins.sync_info
    waits = list(sync_info.on_wait or []) if sync_info is not None else []
    if sync_info is not None and len(waits) > 1:
        sync_info.on_wait = waits[:1]
        for w in waits[1:]:
            n = nc.sync.nop(nofuse=True)
            if n.ins.sync_info is None:
                n.ins.sync_info = type(sync_info)(on_wait=[w], on_update=[])
            else:
                n.ins.sync_info.on_wait = [w]
    nc.sync.drain()
    nc.all_engine_barrier()
    assert self.sems is not None
    popped = nc._tile_sem_poison_stack.pop()
    assert popped is self._sem_poison
    nc.clear_and_free_semaphores(list(self.sems.allocated().values()))
    nc.all_engine_barrier()


tile.TileContext._drain_and_barrier = _patched_drain_and_barrier


# Disk-cache compiled NEFFs by BIR hash — the bass_exec compile path
# bypasses libneuronxla's HLO-keyed cache, so without this every fresh
# process pays the full (~6 min) walrus compile.
def _install_neff_cache():
    import hashlib
    import os
    import shutil

    import concourse.bass2jax as bass2jax

    if getattr(bass2jax, "_neff_cache_installed", False):
        return
    orig = bass2jax.compile_bir_kernel

    def cached(bir_json, tmpdir, neff_name="file.neff"):
        cdir = os.environ.get("BASS_NEFF_CACHE", "/tmp/bass_neff_cache")
        os.makedirs(cdir, exist_ok=True)
        cpath = os.path.join(cdir, hashlib.sha256(bir_json).hexdigest()[:32] + ".neff")
        dst = os.path.join(tmpdir, neff_name)
        if os.path.exists(cpath):
            shutil.copy(cpath, dst)
            return dst
        out = orig(bir_json, tmpdir, neff_name=neff_name)
        try:
            shutil.copy(out, cpath)
        except OSError:
            pass
        return out

    bass2jax.compile_bir_kernel = cached
    bass2jax._neff_cache_installed = True


_install_neff_cache()


def _split_multi_waits(nc: bass.Bass, maxw: int = 1) -> None:
    """Walrus in this container rejects instructions carrying more than one
    semaphore wait. Hoist extra waits onto same-engine no-ops inserted
    right before the instruction (engine streams execute in block order,
    so the waits still gate the instruction)."""
    ctr = 0
    for fn in nc.m.functions:
        for bb in fn.blocks:
            out = []
            changed = False
            for inst in bb.instructions:
                si = inst.sync_info
                waits = list(si.on_wait) if (si is not None and si.on_wait) else []
                if len(waits) > maxw and inst.engine != mybir.EngineType.Unassigned:
                    keep = waits[-maxw:]
                    for i in range(0, len(waits) - maxw, maxw):
                        nop = mybir.InstNoOp(name=f"waitsplit-{ctr}")
                        ctr += 1
                        nop.engine = inst.engine
                        nop.sync_info = mybir.SyncInfo(
                            on_wait=waits[i : i + maxw], on_update=[]
                        )
                        out.append(nop)
                    si.on_wait = keep
                    inst.sync_info = si
                    changed = True
                out.append(inst)
            if changed:
                bb.instructions = out


# ----------------------------------------------------------------------
def _build_nc() -> bass.Bass:
    nc = bass.Bass()

    x_ext = nc.declare_dram_parameter("x", [C, N], F32, isOutput=False)
    w_ext = {
        k: nc.declare_dram_parameter(k, [C, C], F32, isOutput=False)
        for k in ("wqT", "wkT", "wvT", "wpT")
    }
    b_ext = {
        k: nc.declare_dram_parameter(k, [C], F32, isOutput=False)
        for k in ("bq", "bk", "bv", "bp", "gnw", "gnb")
    }
    ind16_ext = nc.declare_dram_parameter("ind16", [128, 8], F32, isOutput=False)
    indT8_ext = nc.declare_dram_parameter("indT8", [8, 128], F32, isOutput=False)
    out_ext = nc.declare_dram_parameter("out", [C, NQ], F32, isOutput=True)

    with tile.TileContext(nc) as tc:
        _body(nc, tc, x_ext, w_ext, b_ext, ind16_ext, indT8_ext, out_ext)
    _split_multi_waits(nc)
    return nc


def _body(nc, tc, x_ext, w_ext, b_ext, ind16_ext, indT8_ext, out_ext):
    from contextlib import ExitStack

    ctx = ExitStack()
    with ctx:
        const = ctx.enter_context(tc.tile_pool(name="const", bufs=1))
        big = ctx.enter_context(tc.tile_pool(name="big", bufs=1))
        mm_psum = ctx.enter_context(tc.tile_pool(name="mm_psum", bufs=3, space="PSUM"))

        # ---- constants -------------------------------------------------
        ind16 = const.tile([128, 8], F32, tag="ind16")
        nc.sync.dma_start(out=ind16, in_=ind16_ext[:])
        indT8 = const.tile([8, 128], F32, tag="indT8")
        nc.sync.dma_start(out=indT8, in_=indT8_ext[:])

        ones_bf = const.tile([128, 1], BF16, tag="ones_bf")
        nc.vector.memset(ones_bf, 1.0)
        ones_row = const.tile([1, 128], F32, tag="ones_row")
        nc.vector.memset(ones_row, 1.0)

        bias_sb = {}
        for k in ("bq", "bk", "bp", "gnw", "gnb"):
            t = const.tile([128, CT], F32, tag=f"bias_{k}")
            nc.sync.dma_start(out=t, in_=b_ext[k][:].rearrange("(o p) -> p o", p=128))
            bias_sb[k] = t
        # bv broadcast along partitions: [512] -> [128, 512]
        bvb = const.tile([128, C], F32, tag="bvb")
        bv_ap = b_ext["bv"][:]
        nc.sync.dma_start(
            out=bvb,
            in_=bass.AP(tensor=bv_ap.tensor, offset=bv_ap.offset, ap=[[0, 128], *bv_ap.ap]),
        )

        # ---- weights: f32 staging -> bf16 [128, CT, C] -----------------
        w_bf = {}
        with tc.tile_pool(name="wstage", bufs=2) as wstage:
            for k in ("wqT", "wkT", "wvT", "wpT"):
                wb = big.tile([128, CT, C], BF16, tag=f"wbf_{k}")
                for ct in range(CT):
                    ws = wstage.tile([128, C], F32, tag="wstage")
                    nc.sync.dma_start(
                        out=ws, in_=w_ext[k][ct * 128 : (ct + 1) * 128, :]
                    )
                    nc.vector.tensor_copy(out=wb[:, ct, :], in_=ws)
                w_bf[k] = wb

        # ---- persistent activations ------------------------------------
        t_big = big.tile([128, CT, N], BF16, tag="t_big")
        xb_big = big.tile([128, CT, NQ], F32, tag="xb_big")

        # ---- phase 1: GroupNorm ----------------------------------------
        with (
            tc.tile_pool(name="gn", bufs=2) as gn_pool,
            tc.tile_pool(name="gn_small", bufs=4) as small,
            tc.tile_pool(name="gn_psum", bufs=2, space="PSUM") as gn_psum,
        ):
            for ct in range(CT):
                xf = gn_pool.tile([128, N], F32, tag="xf")
                nc.sync.dma_start(out=xf, in_=x_ext[ct * 128 : (ct + 1) * 128, :])

                xf3 = xf.rearrange("p (c f) -> p c f", f=512)
                stats6 = small.tile([128, N // 512, 6], F32, tag="stats6")
                for c in range(N // 512):
                    nc.vector.bn_stats(out=stats6[:, c, :], in_=xf3[:, c, :])
                mv = small.tile([128, 2], F32, tag="mv")
                nc.vector.bn_aggr(out=mv, in_=stats6)

                # stats2 = [mean_c, E[x^2]_c]
                stats2 = small.tile([128, 2], F32, tag="stats2")
                nc.vector.tensor_copy(out=stats2[:, 0:1], in_=mv[:, 0:1])
                nc.vector.tensor_mul(stats2[:, 1:2], mv[:, 0:1], mv[:, 0:1])
                nc.vector.tensor_add(stats2[:, 1:2], stats2[:, 1:2], mv[:, 1:2])

                # group aggregation: [8, 2] = (1/16) * sum over 16-ch groups
                gpsum = gn_psum.tile([8, 2], F32, tag="gpsum")
                nc.tensor.matmul(gpsum, lhsT=ind16, rhs=stats2, start=True, stop=True)

                gss = small.tile([8, 2], F32, tag="gss")
                nc.vector.tensor_copy(out=gss, in_=gpsum)
                g_sb = small.tile([8, 2], F32, tag="g_sb")
                nc.vector.tensor_copy(out=g_sb[:, 0:1], in_=gss[:, 0:1])
                msqg = small.tile([8, 1], F32, tag="msqg")
                nc.vector.tensor_mul(msqg, gss[:, 0:1], gss[:, 0:1])
                epsm = small.tile([8, 1], F32, tag="epsm")
                nc.vector.tensor_scalar(
                    epsm,
                    msqg,
                    -1.0,
                    EPS,
                    op0=mybir.AluOpType.mult,
                    op1=mybir.AluOpType.add,
                )
                stdg = small.tile([8, 1], F32, tag="stdg")
                nc.scalar.activation(stdg, gss[:, 1:2], SQRT, bias=epsm, scale=1.0)
                nc.vector.reciprocal(out=g_sb[:, 1:2], in_=stdg)

                # broadcast per-group -> per-channel: [128, 2] = indT8^T @ g_sb
                ppsum = gn_psum.tile([128, 2], F32, tag="ppsum")
                nc.tensor.matmul(ppsum, lhsT=indT8, rhs=g_sb, start=True, stop=True)

                alpha = small.tile([128, 1], F32, tag="alpha")
                nc.vector.tensor_mul(alpha, ppsum[:, 1:2], bias_sb["gnw"][:, ct : ct + 1])
                beta = small.tile([128, 1], F32, tag="beta")
                nc.vector.tensor_mul(beta, ppsum[:, 0:1], alpha)
                nc.vector.tensor_sub(beta, bias_sb["gnb"][:, ct : ct + 1], beta)

                # t = alpha*x + beta (bf16); xb = x + bp (residual staging)
                nc.scalar.activation(t_big[:, ct, :], xf, IDENT, bias=beta, scale=alpha)
                nc.scalar.activation(
                    xb_big[:, ct, :],
                    xf[:, 0:NQ],
                    IDENT,
                    bias=bias_sb["bp"][:, ct : ct + 1],
                    scale=1.0,
                )

        # ---- phase 2: Q / K / V projections ----------------------------
        q_big = big.tile([128, CT, NQ], BF16, tag="q_big")
        k_big = big.tile([128, CT, N], BF16, tag="k_big")
        v_big = big.tile([128, MT, C], BF16, tag="v_big")

        for dt in range(CT):
            for nb in range(QB):
                qp = mm_psum.tile([128, FB], F32, tag="mm")
                for ct in range(CT):
                    nc.tensor.matmul(
                        qp,
                        lhsT=w_bf["wqT"][:, ct, dt * 128 : (dt + 1) * 128],
                        rhs=t_big[:, ct, nb * FB : (nb + 1) * FB],
                        start=(ct == 0),
                        stop=(ct == CT - 1),
                    )
                nc.scalar.activation(
                    q_big[:, dt, nb * FB : (nb + 1) * FB],
                    qp,
                    IDENT,
                    bias=bias_sb["bq"][:, dt : dt + 1],
                    scale=1.0,
                )
        for dt in range(CT):
            for nb in range(NB):
                kp = mm_psum.tile([128, FB], F32, tag="mm")
                for ct in range(CT):
                    nc.tensor.matmul(
                        kp,
                        lhsT=w_bf["wkT"][:, ct, dt * 128 : (dt + 1) * 128],
                        rhs=t_big[:, ct, nb * FB : (nb + 1) * FB],
                        start=(ct == 0),
                        stop=(ct == CT - 1),
                    )
                nc.scalar.activation(
                    k_big[:, dt, nb * FB : (nb + 1) * FB],
                    kp,
                    IDENT,
                    bias=bias_sb["bk"][:, dt : dt + 1],
                    scale=1.0,
                )
        for mt in range(MT):
            vp = mm_psum.tile([128, C], F32, tag="mm")
            for ct in range(CT):
                nc.tensor.matmul(
                    vp,
                    lhsT=t_big[:, ct, mt * 128 : (mt + 1) * 128],
                    rhs=w_bf["wvT"][:, ct, :],
                    start=(ct == 0),
                    stop=(ct == CT - 1),
                )
            nc.vector.tensor_add(v_big[:, mt, :], vp, bvb)

        # ---- phase 3: attention + proj + residual ----------------------
        with (
            tc.tile_pool(name="o_psum", bufs=1, space="PSUM") as o_psum,
            tc.tile_pool(name="lrb_psum", bufs=1, space="PSUM") as lrb_psum,
            tc.tile_pool(name="e_pool", bufs=4) as e_pool,
            tc.tile_pool(name="att_sb", bufs=2) as att_sb,
            tc.tile_pool(name="y_pool", bufs=4) as y_pool,
        ):
            for qb in range(QB):
                qs = slice(qb * FB, (qb + 1) * FB)
                op = [
                    o_psum.tile([128, FB], F32, tag=f"o{dc}", name=f"o_{qb}_{dc}")
                    for dc in range(CT)
                ]
                lp = lrb_psum.tile([128, FB], F32, tag="lrb", name=f"l_{qb}")
                lp1 = lp[0:1, :]

                for mt in range(MT):
                    sp = mm_psum.tile([128, FB], F32, tag="mm")
                    for dt in range(CT):
                        nc.tensor.matmul(
                            sp,
                            lhsT=k_big[:, dt, mt * 128 : (mt + 1) * 128],
                            rhs=q_big[:, dt, qs],
                            start=(dt == 0),
                            stop=(dt == CT - 1),
                        )
                    et = e_pool.tile([128, FB], BF16, tag="et")
                    nc.scalar.activation(et, sp, EXP, bias=0.0, scale=SCALE)
                    nc.tensor.matmul(
                        lp1, lhsT=ones_bf, rhs=et, start=(mt == 0), stop=(mt == MT - 1)
                    )
                    for dc in range(CT):
                        nc.tensor.matmul(
                            op[dc],
                            lhsT=v_big[:, mt, dc * 128 : (dc + 1) * 128],
                            rhs=et,
                            start=(mt == 0),
                            stop=(mt == MT - 1),
                        )

                # evict unnormalized O immediately (frees the psum banks for
                # the next query block without waiting on the 1/L chain)
                o_sb = att_sb.tile([128, CT, FB], BF16, tag="o_sb")
                for dc in range(CT):
                    nc.vector.tensor_copy(out=o_sb[:, dc, :], in_=op[dc])

                # broadcast L across partitions, then full-width reciprocal
                l_sb = att_sb.tile([1, FB], F32, tag="l_sb")
                nc.vector.tensor_copy(out=l_sb, in_=lp1)
                rbp = lrb_psum.tile([128, FB], F32, tag="lrb", name=f"rb_{qb}")
                nc.tensor.matmul(rbp, lhsT=ones_row, rhs=l_sb, start=True, stop=True)
                rb_sb = att_sb.tile([128, FB], F32, tag="rb_sb")
                nc.vector.reciprocal(out=rb_sb, in_=rbp)

                for pt in range(CT):
                    pj = mm_psum.tile([128, FB], F32, tag="mm")
                    for dc in range(CT):
                        nc.tensor.matmul(
                            pj,
                            lhsT=w_bf["wpT"][:, dc, pt * 128 : (pt + 1) * 128],
                            rhs=o_sb[:, dc, :],
                            start=(dc == 0),
                            stop=(dc == CT - 1),
                        )
                    y_tile = y_pool.tile([128, FB], F32, tag="y")
                    nc.vector.tensor_mul(y_tile, pj, rb_sb)
                    nc.vector.tensor_add(y_tile, y_tile, xb_big[:, pt, qs])
                    nc.sync.dma_start(
                        out=out_ext[pt * 128 : (pt + 1) * 128, qs], in_=y_tile
                    )


_NC_CACHE = None


def _get_nc():
    global _NC_CACHE
    if _NC_CACHE is None:
        _NC_CACHE = _build_nc()
    return _NC_CACHE


def _make_indicators():
    p = np.arange(128)
    ind16 = np.zeros((128, 8), np.float32)
    ind16[p, p // GSIZE] = 1.0 / GSIZE
    indT8 = np.zeros((8, 128), np.float32)
    indT8[p // GSIZE, p] = 1.0
    return ind16, indT8


def kernel(**inputs) -> np.ndarray:
    x = np.ascontiguousarray(np.asarray(inputs["x"], dtype=np.float32))
    assert x.shape == (B, C, H, W), x.shape
    xf = x.reshape(B, C, N)

    common = {}
    for name, key in (("wqT", "wq"), ("wkT", "wk"), ("wvT", "wv"), ("wpT", "wp")):
        common[name] = np.ascontiguousarray(np.asarray(inputs[key], np.float32).T)
    for key in ("bq", "bk", "bv", "bp"):
        common[key] = np.ascontiguousarray(np.asarray(inputs[key], np.float32))
    common["gnw"] = np.ascontiguousarray(np.asarray(inputs["gn_w"], np.float32))
    common["gnb"] = np.ascontiguousarray(np.asarray(inputs["gn_b"], np.float32))
    common["ind16"], common["indT8"] = _make_indicators()

    in_maps = []
    for core in range(8):
        b, h = core // 2, core % 2
        if h == 0:
            xc = xf[b]
        else:
            # rotate so this core's query tokens land in columns 0..NQ-1
            xc = np.concatenate([xf[b][:, NQ:], xf[b][:, :NQ]], axis=1)
        in_maps.append({"x": np.ascontiguousarray(xc), **common})

    nc = _get_nc()
    res = run_bass_kernel_spmd(nc, in_maps, core_ids=list(range(8)))

    out = np.empty((B, C, N), np.float32)
    for core in range(8):
        b, h = core // 2, core % 2
        out[b][:, h * NQ : (h + 1) * NQ] = res.results[core]["out"]
    return out.reshape(B, C, H, W)


# revision 16
# speedup vs baseline: 1.7436x; 1.7143x over previous
"""AttentionBlock (GroupNorm + single-head self-attention + residual) on
8 TRN2 NeuronCores.

Sharding: data-parallel over batch (4 images) x 2-way sequence-parallel
over query tokens => 8 cores, zero collectives. Each core receives one
full image x[b] [C=512, N=4096] (token columns rotated so that its own
2048 query tokens sit in columns 0..2047), computes GroupNorm + K/V over
all 4096 tokens (K/V duplicated across the 2 cores of a batch pair --
cheaper than an all-gather at this size), Q / attention / proj / residual
for its 2048 queries, and returns y [512, 2048].

On-chip layout ("channels on partitions"):
  t  = groupnorm(x)            [c, n]  bf16   (4 tiles [128, 4096])
  Q  = wq @ t  (+bq)           [d, nq] bf16
  K  = wk @ t  (+bk)           [d, m]  bf16
  V  = (t^T @ wvT) (+bv)       [m, d]  bf16   (computed directly transposed)
  S^T[m, nq] = K^T Q           (PE, accumulated over 4 d-tiles)
  E  = exp(S^T / sqrt(C))      (ScalarE, PSUM->SBUF, bf16)
  L[nq]  = ones^T @ E          (PE accumulate over 32 m-tiles)
  O_u[d, nq] = V^T @ E         (PE accumulate over 32 m-tiles)
  O  = O_u * (1/L)             (K=1 broadcast matmul + DVE mul)
  y  = wp @ O + bp + x         [c, nq] f32 -> DMA out

Softmax skips the max-subtraction: logits are ~N(0,1) by construction
(1/sqrt(C) scaling of unit-variance q,k), so exp() is safely bounded.
"""

import sys

for _p in ("/opt/trn_rl_repo", "/opt/pypackages"):
    if _p not in sys.path:
        sys.path.append(_p)

import numpy as np

import concourse.bass as bass
import concourse.tile as tile
from concourse import mybir
from concourse.bass_utils import run_bass_kernel_spmd
from concourse.vector_clock import ScopedClock

# ----------------------------------------------------------------------
# Problem constants (nn_AttentionBlock_24764781429183)
B, C, H, W = 4, 512, 64, 64
N = H * W              # 4096 tokens
NQ = N // 2            # 2048 query tokens per core
GROUPS = 32
GSIZE = C // GROUPS    # 16 channels per group
EPS = 1e-5
SCALE = 1.0 / float(np.sqrt(C))
CT = C // 128          # 4 channel tiles
MT = N // 128          # 32 key tiles
FB = 512               # matmul free-dim block
QB = NQ // FB          # 4 query blocks
NB = N // FB           # 8 token blocks

F32 = mybir.dt.float32
BF16 = mybir.dt.bfloat16
FP8 = mybir.dt.float8e4
DR = mybir.MatmulPerfMode.DoubleRow
IDENT = mybir.ActivationFunctionType.Identity
EXP = mybir.ActivationFunctionType.Exp
SQRT = mybir.ActivationFunctionType.Sqrt
# exp(s*SCALE + EXP_SHIFT) = exp(s*SCALE)/8 — keeps E safely inside
# fp8e4m3 range (max 448) even for outlier logits; cancels in E/L.
EXP_SHIFT = -2.0794415416798357


# ----------------------------------------------------------------------
# This container's walrus build rejects >1 semaphore wait on one CTRL
# (Drain) instruction; split the Tile end-of-kernel drain waits across
# one-nop-per-wait instead.
def _patched_drain_and_barrier(self, tick_clock, wait_clock):
    nc = self.nc
    probe = nc.sync.nop(nofuse=True)
    wait_clock.add_sem_waits(probe.ins, ScopedClock({None: tick_clock.global_clock}))
    sync_info = probe.ins.sync_info
    waits = list(sync_info.on_wait or []) if sync_info is not None else []
    if sync_info is not None and len(waits) > 1:
        sync_info.on_wait = waits[:1]
        for w in waits[1:]:
            n = nc.sync.nop(nofuse=True)
            if n.ins.sync_info is None:
                n.ins.sync_info = type(sync_info)(on_wait=[w], on_update=[])
            else:
                n.ins.sync_info.on_wait = [w]
    nc.sync.drain()
    nc.all_engine_barrier()
    assert self.sems is not None
    popped = nc._tile_sem_poison_stack.pop()
    assert popped is self._sem_poison
    nc.clear_and_free_semaphores(list(self.sems.allocated().values()))
    nc.all_engine_barrier()


tile.TileContext._drain_and_barrier = _patched_drain_and_barrier


# Disk-cache compiled NEFFs by BIR hash — the bass_exec compile path
# bypasses libneuronxla's HLO-keyed cache, so without this every fresh
# process pays the full (~6 min) walrus compile.
def _install_neff_cache():
    import hashlib
    import os
    import shutil

    import concourse.bass2jax as bass2jax

    if getattr(bass2jax, "_neff_cache_installed", False):
        return
    orig = bass2jax.compile_bir_kernel

    def cached(bir_json, tmpdir, neff_name="file.neff"):
        cdir = os.environ.get("BASS_NEFF_CACHE", "/tmp/bass_neff_cache")
        os.makedirs(cdir, exist_ok=True)
        cpath = os.path.join(cdir, hashlib.sha256(bir_json).hexdigest()[:32] + ".neff")
        dst = os.path.join(tmpdir, neff_name)
        if os.path.exists(cpath):
            shutil.copy(cpath, dst)
            return dst
        out = orig(bir_json, tmpdir, neff_name=neff_name)
        try:
            shutil.copy(out, cpath)
        except OSError:
            pass
        return out

    bass2jax.compile_bir_kernel = cached
    bass2jax._neff_cache_installed = True


_install_neff_cache()


def _split_multi_waits(nc: bass.Bass, maxw: int = 1) -> None:
    """Walrus in this container rejects instructions carrying more than one
    semaphore wait. Hoist extra waits onto same-engine no-ops inserted
    right before the instruction (engine streams execute in block order,
    so the waits still gate the instruction)."""
    ctr = 0
    for fn in nc.m.functions:
        for bb in fn.blocks:
            out = []
            changed = False
            for inst in bb.instructions:
                si = inst.sync_info
                waits = list(si.on_wait) if (si is not None and si.on_wait) else []
                if len(waits) > maxw and inst.engine != mybir.EngineType.Unassigned:
                    keep = waits[-maxw:]
                    for i in range(0, len(waits) - maxw, maxw):
                        nop = mybir.InstNoOp(name=f"waitsplit-{ctr}")
                        ctr += 1
                        nop.engine = inst.engine
                        nop.sync_info = mybir.SyncInfo(
                            on_wait=waits[i : i + maxw], on_update=[]
                        )
                        out.append(nop)
                    si.on_wait = keep
                    inst.sync_info = si
                    changed = True
                out.append(inst)
            if changed:
                bb.instructions = out


# ----------------------------------------------------------------------
def _build_nc() -> bass.Bass:
    nc = bass.Bass()

    x_ext = nc.declare_dram_parameter("x", [C, N], F32, isOutput=False)
    w_ext = {
        k: nc.declare_dram_parameter(k, [C, C], F32, isOutput=False)
        for k in ("wqT", "wkT", "wvT", "wpT")
    }
    b_ext = {
        k: nc.declare_dram_parameter(k, [C], F32, isOutput=False)
        for k in ("bq", "bk", "bv", "bp", "gnw", "gnb")
    }
    ind16_ext = nc.declare_dram_parameter("ind16", [128, 8], F32, isOutput=False)
    indT8_ext = nc.declare_dram_parameter("indT8", [8, 128], F32, isOutput=False)
    out_ext = nc.declare_dram_parameter("out", [C, NQ], F32, isOutput=True)

    with tile.TileContext(nc) as tc:
        _body(nc, tc, x_ext, w_ext, b_ext, ind16_ext, indT8_ext, out_ext)
    _split_multi_waits(nc)
    return nc


def _body(nc, tc, x_ext, w_ext, b_ext, ind16_ext, indT8_ext, out_ext):
    from contextlib import ExitStack

    ctx = ExitStack()
    with ctx:
        const = ctx.enter_context(tc.tile_pool(name="const", bufs=1))
        big = ctx.enter_context(tc.tile_pool(name="big", bufs=1))
        mm_psum = ctx.enter_context(tc.tile_pool(name="mm_psum", bufs=3, space="PSUM"))

        # ---- constants -------------------------------------------------
        ind16 = const.tile([128, 8], F32, tag="ind16")
        nc.sync.dma_start(out=ind16, in_=ind16_ext[:])
        indT8 = const.tile([8, 128], F32, tag="indT8")
        nc.sync.dma_start(out=indT8, in_=indT8_ext[:])

        # DoubleRow lhsT needs the pair-dim step to be 16B-aligned, so pad
        # the ones column out to 16 and slice.
        ones_dr_full = const.tile([128, 2, 16], FP8, tag="ones_dr")
        nc.vector.memset(ones_dr_full, 1.0)
        ones_dr = ones_dr_full[:, :, 0:1]
        ones_row = const.tile([1, 128], F32, tag="ones_row")
        nc.vector.memset(ones_row, 1.0)
        expshift = const.tile([128, 1], F32, tag="expshift")
        nc.vector.memset(expshift, EXP_SHIFT)

        bias_sb = {}
        for k in ("bq", "bk", "bp", "gnw", "gnb"):
            t = const.tile([128, CT], F32, tag=f"bias_{k}")
            nc.sync.dma_start(out=t, in_=b_ext[k][:].rearrange("(o p) -> p o", p=128))
            bias_sb[k] = t
        # bv broadcast along partitions: [512] -> [128, 512]
        bvb = const.tile([128, C], F32, tag="bvb")
        bv_ap = b_ext["bv"][:]
        nc.sync.dma_start(
            out=bvb,
            in_=bass.AP(tensor=bv_ap.tensor, offset=bv_ap.offset, ap=[[0, 128], *bv_ap.ap]),
        )

        # ---- weights: f32 staging -> bf16 [128, CT, C] -----------------
        w_bf = {}
        with tc.tile_pool(name="wstage", bufs=2) as wstage:
            for k in ("wqT", "wkT", "wvT", "wpT"):
                wb = big.tile([128, CT, C], BF16, tag=f"wbf_{k}")
                for ct in range(CT):
                    ws = wstage.tile([128, C], F32, tag="wstage")
                    nc.sync.dma_start(
                        out=ws, in_=w_ext[k][ct * 128 : (ct + 1) * 128, :]
                    )
                    nc.vector.tensor_copy(out=wb[:, ct, :], in_=ws)
                w_bf[k] = wb

        # ---- persistent activations ------------------------------------
        t_big = big.tile([128, CT, N], BF16, tag="t_big")
        xb_big = big.tile([128, CT, NQ], F32, tag="xb_big")

        # ---- phase 1: GroupNorm ----------------------------------------
        with (
            tc.tile_pool(name="gn", bufs=2) as gn_pool,
            tc.tile_pool(name="gn_small", bufs=4) as small,
            tc.tile_pool(name="gn_psum", bufs=2, space="PSUM") as gn_psum,
        ):
            for ct in range(CT):
                xf = gn_pool.tile([128, N], F32, tag="xf")
                nc.sync.dma_start(out=xf, in_=x_ext[ct * 128 : (ct + 1) * 128, :])

                xf3 = xf.rearrange("p (c f) -> p c f", f=512)
                stats6 = small.tile([128, N // 512, 6], F32, tag="stats6")
                for c in range(N // 512):
                    nc.vector.bn_stats(out=stats6[:, c, :], in_=xf3[:, c, :])
                mv = small.tile([128, 2], F32, tag="mv")
                nc.vector.bn_aggr(out=mv, in_=stats6)

                # stats2 = [mean_c, E[x^2]_c]
                stats2 = small.tile([128, 2], F32, tag="stats2")
                nc.vector.tensor_copy(out=stats2[:, 0:1], in_=mv[:, 0:1])
                nc.vector.tensor_mul(stats2[:, 1:2], mv[:, 0:1], mv[:, 0:1])
                nc.vector.tensor_add(stats2[:, 1:2], stats2[:, 1:2], mv[:, 1:2])

                # group aggregation: [8, 2] = (1/16) * sum over 16-ch groups
                gpsum = gn_psum.tile([8, 2], F32, tag="gpsum")
                nc.tensor.matmul(gpsum, lhsT=ind16, rhs=stats2, start=True, stop=True)

                gss = small.tile([8, 2], F32, tag="gss")
                nc.vector.tensor_copy(out=gss, in_=gpsum)
                g_sb = small.tile([8, 2], F32, tag="g_sb")
                nc.vector.tensor_copy(out=g_sb[:, 0:1], in_=gss[:, 0:1])
                msqg = small.tile([8, 1], F32, tag="msqg")
                nc.vector.tensor_mul(msqg, gss[:, 0:1], gss[:, 0:1])
                epsm = small.tile([8, 1], F32, tag="epsm")
                nc.vector.tensor_scalar(
                    epsm,
                    msqg,
                    -1.0,
                    EPS,
                    op0=mybir.AluOpType.mult,
                    op1=mybir.AluOpType.add,
                )
                stdg = small.tile([8, 1], F32, tag="stdg")
                nc.scalar.activation(stdg, gss[:, 1:2], SQRT, bias=epsm, scale=1.0)
                nc.vector.reciprocal(out=g_sb[:, 1:2], in_=stdg)

                # broadcast per-group -> per-channel: [128, 2] = indT8^T @ g_sb
                ppsum = gn_psum.tile([128, 2], F32, tag="ppsum")
                nc.tensor.matmul(ppsum, lhsT=indT8, rhs=g_sb, start=True, stop=True)

                alpha = small.tile([128, 1], F32, tag="alpha")
                nc.vector.tensor_mul(alpha, ppsum[:, 1:2], bias_sb["gnw"][:, ct : ct + 1])
                beta = small.tile([128, 1], F32, tag="beta")
                nc.vector.tensor_mul(beta, ppsum[:, 0:1], alpha)
                nc.vector.tensor_sub(beta, bias_sb["gnb"][:, ct : ct + 1], beta)

                # t = alpha*x + beta (bf16); xb = x + bp (residual staging)
                nc.scalar.activation(t_big[:, ct, :], xf, IDENT, bias=beta, scale=alpha)
                nc.scalar.activation(
                    xb_big[:, ct, :],
                    xf[:, 0:NQ],
                    IDENT,
                    bias=bias_sb["bp"][:, ct : ct + 1],
                    scale=1.0,
                )

        # ---- phase 2: Q / K / V projections (outputs in fp8 for the
        # DoubleRow attention matmuls) ----------------------------------
        q_big = big.tile([128, CT, NQ], FP8, tag="q_big")
        k_big = big.tile([128, CT, N], FP8, tag="k_big")
        v_big = big.tile([128, MT, C], FP8, tag="v_big")

        for dt in range(CT):
            for nb in range(QB):
                qp = mm_psum.tile([128, FB], F32, tag="mm")
                for ct in range(CT):
                    nc.tensor.matmul(
                        qp,
                        lhsT=w_bf["wqT"][:, ct, dt * 128 : (dt + 1) * 128],
                        rhs=t_big[:, ct, nb * FB : (nb + 1) * FB],
                        start=(ct == 0),
                        stop=(ct == CT - 1),
                    )
                nc.vector.tensor_scalar_add(
                    q_big[:, dt, nb * FB : (nb + 1) * FB],
                    qp,
                    bias_sb["bq"][:, dt : dt + 1],
                )
        for dt in range(CT):
            for nb in range(NB):
                kp = mm_psum.tile([128, FB], F32, tag="mm")
                for ct in range(CT):
                    nc.tensor.matmul(
                        kp,
                        lhsT=w_bf["wkT"][:, ct, dt * 128 : (dt + 1) * 128],
                        rhs=t_big[:, ct, nb * FB : (nb + 1) * FB],
                        start=(ct == 0),
                        stop=(ct == CT - 1),
                    )
                nc.vector.tensor_scalar_add(
                    k_big[:, dt, nb * FB : (nb + 1) * FB],
                    kp,
                    bias_sb["bk"][:, dt : dt + 1],
                )
        for mt in range(MT):
            vp = mm_psum.tile([128, C], F32, tag="mm")
            for ct in range(CT):
                nc.tensor.matmul(
                    vp,
                    lhsT=t_big[:, ct, mt * 128 : (mt + 1) * 128],
                    rhs=w_bf["wvT"][:, ct, :],
                    start=(ct == 0),
                    stop=(ct == CT - 1),
                )
            nc.vector.tensor_add(v_big[:, mt, :], vp, bvb)

        # ---- phase 3: attention + proj + residual ----------------------
        with (
            tc.tile_pool(name="o_psum", bufs=1, space="PSUM") as o_psum,
            tc.tile_pool(name="lrb_psum", bufs=1, space="PSUM") as lrb_psum,
            tc.tile_pool(name="e_pool", bufs=4) as e_pool,
            tc.tile_pool(name="att_sb", bufs=2) as att_sb,
            tc.tile_pool(name="y_pool", bufs=4) as y_pool,
        ):
            for qb in range(QB):
                qs = slice(qb * FB, (qb + 1) * FB)
                op = [
                    o_psum.tile([128, FB], F32, tag=f"o{dc}", name=f"o_{qb}_{dc}")
                    for dc in range(CT)
                ]
                lp = lrb_psum.tile([128, FB], F32, tag="lrb", name=f"l_{qb}")
                lp1 = lp[0:1, :]

                for pr in range(MT // 2):  # pairs of key tiles (DoubleRow K=256)
                    etp = e_pool.tile([128, 2, FB], FP8, tag="etp")
                    for half in range(2):
                        mt = 2 * pr + half
                        sp = mm_psum.tile([128, FB], F32, tag="mm")
                        for dt2 in range(CT // 2):
                            nc.tensor.matmul(
                                sp,
                                lhsT=k_big[:, 2 * dt2 : 2 * dt2 + 2, mt * 128 : (mt + 1) * 128],
                                rhs=q_big[:, 2 * dt2 : 2 * dt2 + 2, qs],
                                start=(dt2 == 0),
                                stop=(dt2 == CT // 2 - 1),
                                perf_mode=DR,
                            )
                        nc.scalar.activation(
                            etp[:, half, :], sp, EXP, bias=expshift, scale=SCALE
                        )
                    nc.tensor.matmul(
                        lp1,
                        lhsT=ones_dr,
                        rhs=etp,
                        start=(pr == 0),
                        stop=(pr == MT // 2 - 1),
                        perf_mode=DR,
                    )
                    for dc in range(CT):
                        nc.tensor.matmul(
                            op[dc],
                            lhsT=v_big[:, 2 * pr : 2 * pr + 2, dc * 128 : (dc + 1) * 128],
                            rhs=etp,
                            start=(pr == 0),
                            stop=(pr == MT // 2 - 1),
                            perf_mode=DR,
                        )

                # evict unnormalized O immediately (frees the psum banks for
                # the next query block without waiting on the 1/L chain)
                o_sb = att_sb.tile([128, CT, FB], BF16, tag="o_sb")
                for dc in range(CT):
                    nc.vector.tensor_copy(out=o_sb[:, dc, :], in_=op[dc])

                # broadcast L across partitions, then full-width reciprocal
                l_sb = att_sb.tile([1, FB], F32, tag="l_sb")
                nc.vector.tensor_copy(out=l_sb, in_=lp1)
                rbp = lrb_psum.tile([128, FB], F32, tag="lrb", name=f"rb_{qb}")
                nc.tensor.matmul(rbp, lhsT=ones_row, rhs=l_sb, start=True, stop=True)
                rb_sb = att_sb.tile([128, FB], F32, tag="rb_sb")
                nc.vector.reciprocal(out=rb_sb, in_=rbp)

                for pt in range(CT):
                    pj = mm_psum.tile([128, FB], F32, tag="mm")
                    for dc in range(CT):
                        nc.tensor.matmul(
                            pj,
                            lhsT=w_bf["wpT"][:, dc, pt * 128 : (pt + 1) * 128],
                            rhs=o_sb[:, dc, :],
                            start=(dc == 0),
                            stop=(dc == CT - 1),
                        )
                    y_tile = y_pool.tile([128, FB], F32, tag="y")
                    nc.vector.tensor_mul(y_tile, pj, rb_sb)
                    nc.vector.tensor_add(y_tile, y_tile, xb_big[:, pt, qs])
                    nc.sync.dma_start(
                        out=out_ext[pt * 128 : (pt + 1) * 128, qs], in_=y_tile
                    )


_NC_CACHE = None


def _get_nc():
    global _NC_CACHE
    if _NC_CACHE is None:
        _NC_CACHE = _build_nc()
    return _NC_CACHE


def _make_indicators():
    p = np.arange(128)
    ind16 = np.zeros((128, 8), np.float32)
    ind16[p, p // GSIZE] = 1.0 / GSIZE
    indT8 = np.zeros((8, 128), np.float32)
    indT8[p // GSIZE, p] = 1.0
    return ind16, indT8


def kernel(**inputs) -> np.ndarray:
    x = np.ascontiguousarray(np.asarray(inputs["x"], dtype=np.float32))
    assert x.shape == (B, C, H, W), x.shape
    xf = x.reshape(B, C, N)

    common = {}
    for name, key in (("wqT", "wq"), ("wkT", "wk"), ("wvT", "wv"), ("wpT", "wp")):
        common[name] = np.ascontiguousarray(np.asarray(inputs[key], np.float32).T)
    for key in ("bq", "bk", "bv", "bp"):
        common[key] = np.ascontiguousarray(np.asarray(inputs[key], np.float32))
    common["gnw"] = np.ascontiguousarray(np.asarray(inputs["gn_w"], np.float32))
    common["gnb"] = np.ascontiguousarray(np.asarray(inputs["gn_b"], np.float32))
    common["ind16"], common["indT8"] = _make_indicators()

    in_maps = []
    for core in range(8):
        b, h = core // 2, core % 2
        if h == 0:
            xc = xf[b]
        else:
            # rotate so this core's query tokens land in columns 0..NQ-1
            xc = np.concatenate([xf[b][:, NQ:], xf[b][:, :NQ]], axis=1)
        in_maps.append({"x": np.ascontiguousarray(xc), **common})

    nc = _get_nc()
    res = run_bass_kernel_spmd(nc, in_maps, core_ids=list(range(8)))

    out = np.empty((B, C, N), np.float32)
    for core in range(8):
        b, h = core // 2, core % 2
        out[b][:, h * NQ : (h + 1) * NQ] = res.results[core]["out"]
    return out.reshape(B, C, H, W)
